# revision 1
# baseline (speedup 1.0000x reference)
"""Trainium2 Bass kernel for BaseLUTLayer (probabilistic LUT node eval).

Math (per reference):
  x_eff = where(flip, 1 - x, x)                      # (B, IN)
  g[b,n,j] = x_eff[b, mapping[n,j]]                  # gather, (B, N, 6)
  out[b,n] = sum_k sigmoid(lut[n,k]) * prod_j (g_j if bit_j(k) else 1-g_j)

Evaluated on-device as a 6-level multilinear contraction per (b, n):
  level 0 folds the LSB of the 64-entry sigmoid table with per-node
  scalars (tensor_scalar FMA, per-partition scalar operands), levels 1-5
  are lerps V = U_even + a_j * (U_odd - U_even) done with tensor_tensor
  ops and a 0-stride broadcast AP for a_j.

Sharding: nodes split 8 ways (1024 nodes/core); batch replicated.
x and flip are host-transposed to (IN, B) so dma_gather (the SWDGE
embedding-lookup primitive) can fetch one 256-float row per (node, fanin)
index.  Per-core output is (1024, 256), host concatenates + transposes.
"""

import numpy as np

B = 256
IN = 8192
NN = 8192
FAN = 6
NPAT = 64
NCORES = 8
PT = 128  # nodes per tile (partition dim)

_CACHE = {}


def _build_nc(nl, b, inp, fp16=True):
    """Build + compile the SPMD Bass program for one core's slice.

    nl: nodes per core, b: batch (replicated), inp: input size.
    """
    import concourse.bacc as bacc
    import concourse.mybir as mybir
    from concourse.tile import TileContext
    from concourse._compat import get_trn_type

    dt = mybir.dt
    Alu = mybir.AluOpType
    Act = mybir.ActivationFunctionType

    nt = nl // PT
    n_idx = nl * FAN          # gather indices total
    n_idx_t = PT * FAN        # per tile (768)
    iw = n_idx // 16          # idx wrap columns

    nc = bacc.Bacc(
        get_trn_type() or "TRN2",
        target_bir_lowering=False,
        debug=False,
        num_devices=NCORES,
    )
    # merged gather table: per input row, 2*b bytes of fp16 x then b bytes of u8 flip
    rowb = 3 * b
    xfT = nc.dram_tensor("xfT", [inp, rowb], dt.uint8, kind="ExternalInput")
    lut = nc.dram_tensor("lut", [nl, NPAT], dt.float32, kind="ExternalInput")
    idx = nc.dram_tensor("idx", [128, iw], dt.int16, kind="ExternalInput")
    outT = nc.dram_tensor("outT", [nl, b], dt.float32, kind="ExternalOutput")

    cdt = dt.float16 if fp16 else dt.float32

    with TileContext(nc) as tc:
        with (
            tc.tile_pool(name="const", bufs=1) as cpool,
            tc.tile_pool(name="ld", bufs=2) as ld,
            tc.tile_pool(name="small", bufs=3) as sm,
            tc.tile_pool(name="work", bufs=2) as wk,
        ):
            idx_sb = cpool.tile([128, iw], dt.int16)
            nc.sync.dma_start(idx_sb[:, :], idx[:, :])

            for t in range(nt):
                # --- loads: one gather brings x (fp16) + flip (u8) rows ---
                g = ld.tile([128, FAN, rowb], dt.uint8, tag="g")
                nc.gpsimd.dma_gather(
                    g[:, :, :], xfT[:, :], idx_sb[:, t * (n_idx_t // 16):(t + 1) * (n_idx_t // 16)],
                    n_idx_t, n_idx_t, rowb,
                )
                xg = g[:, :, 0:2 * b].bitcast(dt.float16)
                fg = g[:, :, 2 * b:rowb]
                lut_t = ld.tile([128, NPAT], dt.float32, tag="lut")
                nc.sync.dma_start(lut_t[:, :], lut[t * PT:(t + 1) * PT, :])

                # --- per-node table prep (Pool: small 2-input subs) ---
                # sig[k] = sigmoid(lut[k]); d0[m] = sig[2m+1]-sig[2m]
                # dE[q] = sig[4q+2]-sig[4q]; dD[q] = d0[2q+1]-d0[2q]
                sig = sm.tile([128, NPAT], dt.float32, tag="sig")
                nc.scalar.activation(sig[:, :], lut_t[:, :], Act.Sigmoid)
                d0 = sm.tile([128, NPAT // 2], dt.float32, tag="d0")
                nc.gpsimd.tensor_sub(d0[:, :], sig[:, 1::2], sig[:, 0::2])
                dE = sm.tile([128, NPAT // 4], dt.float32, tag="dE")
                nc.gpsimd.tensor_sub(dE[:, :], sig[:, 2::4], sig[:, 0::4])
                dD = sm.tile([128, NPAT // 4], dt.float32, tag="dD")
                nc.gpsimd.tensor_sub(dD[:, :], d0[:, 1::2], d0[:, 0::2])

                # --- flip: x_eff = |f - x|  (exact for f in {0,1}) ---
                # fanin 0 first (short critical path into level 0), 1-5 after
                ff = sm.tile([128, FAN, b], cdt, tag="ff")
                nc.scalar.activation(ff[:, :, :], fg[:, :, :], Act.Copy)
                dfx = sm.tile([128, FAN, b], cdt, tag="dfx")
                xe = sm.tile([128, FAN, b], cdt, tag="xe")
                nc.vector.tensor_sub(dfx[:, 0, :], ff[:, 0, :], xg[:, 0, :])
                nc.vector.tensor_sub(dfx[:, 1:, :], ff[:, 1:, :], xg[:, 1:, :])
                nc.scalar.activation(xe[:, 0, :], dfx[:, 0, :], Act.Abs)
                nc.scalar.activation(xe[:, 1:, :], dfx[:, 1:, :], Act.Abs)

                # --- level 0+1a: Ue[q] = sig[4q] + a0*d0[2q]
                #                 D1[q] = dE[q] + a0*dD[q]
                # 32 per-partition-scalar FMAs split across ACT/Pool/DVE, in
                # 4 q-chunks; level 1b (V1 = Ue + a1*D1) issued per chunk so
                # DVE starts before the whole level-0 sweep finishes.
                a0 = xe[:, 0, :]
                a1c = xe[:, 1:2, :]
                Ue = wk.tile([128, 16, b], cdt, tag="Ue")
                D1 = wk.tile([128, 16, b], cdt, tag="D1")
                P1 = wk.tile([128, 16, b], cdt, tag="P1")
                V = wk.tile([128, 16, b], cdt, tag="V1")
                # per chunk of 4 q's: 8 jobs; DVE-heavy on the ramp tile,
                # ACT/Pool-heavy in steady state
                if t == 0:
                    homes = ["dve", "dve", "dve", "dve", "dve", "dve", "pool", "pool"]
                elif t == 1:
                    homes = ["act", "act", "act", "dve", "dve", "pool", "pool", "pool"]
                else:
                    homes = ["act", "act", "act", "act", "act", "pool", "pool", "pool"]
                for c4 in range(4):
                    for i in range(8):
                        q = c4 * 4 + i // 2
                        if i % 2 == 0:
                            dst, sc, bi = Ue[:, q, :], d0[:, 2 * q:2 * q + 1], sig[:, 4 * q:4 * q + 1]
                        else:
                            dst, sc, bi = D1[:, q, :], dD[:, q:q + 1], dE[:, q:q + 1]
                        h = homes[i]
                        if h == "act":
                            nc.scalar.activation(dst, a0, Act.Identity, scale=sc, bias=bi)
                        elif h == "pool":
                            nc.gpsimd.tensor_scalar(
                                out=dst, in0=a0, scalar1=sc, scalar2=bi,
                                op0=Alu.mult, op1=Alu.add,
                            )
                        else:
                            nc.vector.tensor_scalar(
                                out=dst, in0=a0, scalar1=sc, scalar2=bi,
                                op0=Alu.mult, op1=Alu.add,
                            )
                    qs = slice(c4 * 4, c4 * 4 + 4)
                    a1 = a1c.broadcast_to([128, 4, b])
                    nc.vector.tensor_mul(P1[:, qs, :], D1[:, qs, :], a1)
                    nc.vector.tensor_add(V[:, qs, :], P1[:, qs, :], Ue[:, qs, :])

                # --- levels 2..5: V = U_e + a_j*(U_o - U_e) ---
                for j in range(2, 6):
                    h = 32 >> j  # output pattern count
                    eng = nc.gpsimd if (j == 5 and t != nt - 1) else nc.vector
                    D = wk.tile([128, h, b], cdt, tag=f"D{j}")
                    eng.tensor_sub(D[:, :, :], V[:, 1::2, :], V[:, 0::2, :])
                    a = xe[:, j:j + 1, :].broadcast_to([128, h, b])
                    P = wk.tile([128, h, b], cdt, tag=f"P{j}")
                    eng.tensor_mul(P[:, :, :], D[:, :, :], a)
                    odt = dt.float32 if j == 5 else cdt
                    Vn = wk.tile([128, h, b], odt, tag=f"V{j}")
                    eng.tensor_add(Vn[:, :, :], P[:, :, :], V[:, 0::2, :])
                    V = Vn

                nc.sync.dma_start(outT[t * PT:(t + 1) * PT, :], V[:, 0, :])

    nc.compile()
    return nc


def _prep_core_inputs(x, lut_table, mapping, flip_mask, nl, b, inp, n_cores=NCORES):
    """Host-side layout prep (pure data movement): transpose + slice + index pack."""
    xf = np.empty((inp, 3 * b), np.uint8)                          # (IN, 3B)
    xf[:, :2 * b] = np.ascontiguousarray(x.T, dtype=np.float16).view(np.uint8)
    xf[:, 2 * b:] = np.ascontiguousarray(flip_mask.T).astype(np.uint8)
    nt = nl // PT
    in_maps = []
    for c in range(n_cores):
        sl = slice(c * nl, (c + 1) * nl)
        lut_c = np.ascontiguousarray(lut_table[sl], dtype=np.float32)
        m_c = np.asarray(mapping[sl])                              # (nl, 6) int32
        # gather order: j = (t*6+f)*128 + p  ->  m_c[t*128+p, f]
        order = m_c.reshape(nt, PT, FAN).transpose(0, 2, 1).reshape(-1)
        idx16 = order.astype(np.int16)
        wrapped = np.ascontiguousarray(idx16.reshape(-1, 16).T)    # (16, nl*6/16)
        idx_full = np.tile(wrapped, (8, 1))                        # (128, ...)
        in_maps.append({"xfT": xf, "lut": lut_c, "idx": idx_full})
    return in_maps


def _run(nc, in_maps, **kw):
    from concourse.bass_utils import run_bass_kernel_spmd

    last = None
    for attempt in range(3):
        try:
            return run_bass_kernel_spmd(nc, in_maps, list(range(NCORES)), **kw)
        except Exception as e:  # transient device errors happen on this fabric
            last = e
            if "UNRECOVERABLE" not in str(e) and "UNAVAILABLE" not in str(e):
                raise
    raise last


def kernel(x, lut_table, mapping, flip_mask):
    b, inp = x.shape
    nn = lut_table.shape[0]
    nl = nn // NCORES
    key = (nl, b, inp)
    if key not in _CACHE:
        _CACHE[key] = _build_nc(nl, b, inp)
    nc = _CACHE[key]
    in_maps = _prep_core_inputs(x, lut_table, mapping, flip_mask, nl, b, inp)
    res = _run(nc, in_maps)
    outT = np.concatenate([res.results[c]["outT"] for c in range(NCORES)], axis=0)
    return np.ascontiguousarray(outT.T, dtype=np.float32)



# revision 4
# speedup vs baseline: 1.1718x; 1.1718x over previous
"""Trainium2 Bass kernel for BaseLUTLayer (probabilistic LUT node eval).

Math (per reference):
  x_eff = where(flip, 1 - x, x)                      # (B, IN)
  g[b,n,j] = x_eff[b, mapping[n,j]]                  # gather, (B, N, 6)
  out[b,n] = sum_k sigmoid(lut[n,k]) * prod_j (g_j if bit_j(k) else 1-g_j)

Device algorithm (centered-monomial basis):
  host:  t[b,i] = (x[b,i] - 0.5) * (1 - 2*flip[b,i])          (fp16, (IN,B))
         C[n,:] = centered-monomial transform of sigmoid(lut[n,:])
                  (out = sum_S C[n,S] * prod_{j in S} t_j, |t_j| <= 0.5)
  dev:   gather the 6 t-rows per node (dma_gather), then fold:
           level 0:  U[m] = C[2m] + t0 * C[2m+1]     32 scalar-FMA rows
                     (per-partition fp32 scalar operands -> DVE 4x / ACT / Pool)
           level j:  V = U_even + t_j * U_odd        mul+add tensor rows,
                     batch columns split between a DVE lane and a Pool lane.

Sharding: nodes split 8 ways (1024 nodes/core); batch replicated.
Per-core output is (1024, 256) fp32, host concatenates + transposes.
"""

import numpy as np

B = 256
IN = 8192
NN = 8192
FAN = 6
NPAT = 64
NCORES = 8
PT = 128  # nodes per tile (partition dim)

# engine split tuning (see _build_nc)
N_ACT = 24   # level-0 FMA rows on ACT (of 32)
N_POOL = 0   # level-0 FMA rows on Pool
W_DVE = 208  # batch columns of levels 1-5 on DVE (rest on Pool)
NG = 2       # number of gather chunks

_CACHE = {}


def _build_nc(nl, b, inp, n_act=N_ACT, n_pool=N_POOL, w_dve=W_DVE, ng=NG):
    """Build + compile the SPMD Bass program for one core's slice."""
    import concourse.bacc as bacc
    import concourse.mybir as mybir
    from concourse.tile import TileContext
    from concourse._compat import get_trn_type

    dt = mybir.dt
    Alu = mybir.AluOpType
    Act = mybir.ActivationFunctionType

    nt = nl // PT              # tiles
    tpg = nt // ng             # tiles per gather
    npg = PT * FAN * tpg       # indices per gather
    iw = npg // 16             # idx wrap columns per gather

    nc = bacc.Bacc(
        get_trn_type() or "TRN2",
        target_bir_lowering=False,
        debug=False,
        num_devices=NCORES,
    )
    tT = nc.dram_tensor("tT", [inp, b], dt.float16, kind="ExternalInput")
    Ctab = nc.dram_tensor("C", [nl, NPAT], dt.float32, kind="ExternalInput")
    idx = nc.dram_tensor("idx", [128, ng * iw], dt.int16, kind="ExternalInput")
    outT = nc.dram_tensor("outT", [nl, b], dt.float32, kind="ExternalOutput")

    f16, f32 = dt.float16, dt.float32
    wP = b - w_dve

    with TileContext(nc) as tc:
        with (
            tc.tile_pool(name="const", bufs=1) as cpool,
            tc.tile_pool(name="ld", bufs=2) as ld,
            tc.tile_pool(name="work", bufs=2) as wk,
        ):
            idx_sb = cpool.tile([128, ng * iw], dt.int16)
            nc.sync.dma_start(idx_sb[:, :], idx[:, :])

            gt = []
            for G in range(ng):
                g = cpool.tile([128, tpg * FAN, b], f16, tag=f"g{G}")
                nc.gpsimd.dma_gather(
                    g[:, :, :], tT[:, :], idx_sb[:, G * iw:(G + 1) * iw],
                    npg, npg, b,
                )
                gt.append(g)

            for t in range(nt):
                G, tl = divmod(t, tpg)
                a = lambda j: gt[G][:, tl * FAN + j, :]
                a3 = lambda j, sl: gt[G][:, tl * FAN + j:tl * FAN + j + 1, sl]

                Ct = ld.tile([128, NPAT], f32, tag="C")
                nc.sync.dma_start(Ct[:, :], Ctab[t * PT:(t + 1) * PT, :])

                # --- level 0: U[m] = C[2m] + t0*C[2m+1], 32 scalar-FMA rows ---
                U = wk.tile([128, 32, b], f16, tag="U")
                t0 = a(0)
                for m in range(32):
                    dst = U[:, m, :]
                    sc, bi = Ct[:, 2 * m + 1:2 * m + 2], Ct[:, 2 * m:2 * m + 1]
                    if m < n_act:
                        nc.scalar.activation(dst, t0, Act.Identity, scale=sc, bias=bi)
                    elif m < n_act + n_pool:
                        nc.gpsimd.tensor_scalar(
                            out=dst, in0=t0, scalar1=sc, scalar2=bi,
                            op0=Alu.mult, op1=Alu.add,
                        )
                    else:
                        nc.vector.tensor_scalar(
                            out=dst, in0=t0, scalar1=sc, scalar2=bi,
                            op0=Alu.mult, op1=Alu.add,
                        )

                # --- levels 1..5: V = U_even + t_j*U_odd, two column lanes ---
                out_t = wk.tile([128, 1, b], f32, tag="out")
                lanes = []
                if w_dve > 0:
                    lanes.append((nc.vector, slice(0, w_dve), w_dve, "D"))
                if wP > 0:
                    lanes.append((nc.gpsimd, slice(w_dve, b), wP, "P"))
                for eng, sl, w, nm in lanes:
                    V = U[:, :, sl]
                    for j in range(1, 6):
                        h = 32 >> j
                        tjb = a3(j, sl).broadcast_to([128, h, w])
                        P = wk.tile([128, h, w], f16, tag=f"P{j}{nm}")
                        eng.tensor_mul(P[:, :, :], V[:, 1::2, :], tjb)
                        if j < 5:
                            Vn = wk.tile([128, h, w], f16, tag=f"V{j}{nm}")
                            eng.tensor_add(Vn[:, :, :], P[:, :, :], V[:, 0::2, :])
                            V = Vn
                        else:
                            eng.tensor_add(
                                out_t[:, :, sl], P[:, :, :], V[:, 0::2, :],
                            )

                nc.sync.dma_start(outT[t * PT:(t + 1) * PT, :], out_t[:, 0, :])

    nc.compile()
    return nc


def _prep_core_inputs(x, lut_table, mapping, flip_mask, nl, b, inp, n_cores=NCORES, ng=NG):
    """Host-side layout prep: t-table, centered-monomial tables, packed indices."""
    x = np.asarray(x)
    flip = np.asarray(flip_mask)
    # t[b,i] = (x-0.5)*(1-2f), transposed to (IN, B) fp16 for the gather
    tT = np.ascontiguousarray(
        ((x - 0.5) * (1.0 - 2.0 * flip)).T.astype(np.float16)
    )

    # centered-monomial transform of sigmoid(lut): out = sum_S C_S prod_{j in S} t_j
    lut64 = np.asarray(lut_table, dtype=np.float64)
    s = 1.0 / (1.0 + np.exp(-lut64))
    C = s.reshape(-1, 2, 2, 2, 2, 2, 2)  # axes [N, b5, b4, b3, b2, b1, b0]
    for j in range(6):
        ax = 1 + (5 - j)
        e = np.take(C, 0, axis=ax)
        o = np.take(C, 1, axis=ax)
        C = np.stack([0.5 * (e + o), o - e], axis=ax)
    C = np.ascontiguousarray(C.reshape(-1, NPAT), dtype=np.float32)

    nt = nl // PT
    tpg = nt // ng
    in_maps = []
    for c in range(n_cores):
        sl = slice(c * nl, (c + 1) * nl)
        m_c = np.asarray(mapping[sl])  # (nl, 6) int32
        # per gather G: local index j = (tl*6+f)*128 + p -> m_c[(G*tpg+tl)*128+p, f]
        order = m_c.reshape(ng, tpg, PT, FAN).transpose(0, 1, 3, 2).reshape(ng, -1)
        wraps = []
        for G in range(ng):
            w = np.ascontiguousarray(order[G].astype(np.int16).reshape(-1, 16).T)
            wraps.append(np.tile(w, (8, 1)))  # (128, iw)
        idx_full = np.concatenate(wraps, axis=1)
        in_maps.append({"tT": tT, "C": np.ascontiguousarray(C[sl]), "idx": idx_full})
    return in_maps


def _run(nc, in_maps, **kw):
    from concourse.bass_utils import run_bass_kernel_spmd

    last = None
    for attempt in range(3):
        try:
            return run_bass_kernel_spmd(nc, in_maps, list(range(NCORES)), **kw)
        except Exception as e:  # transient device errors happen on this fabric
            last = e
            if "UNRECOVERABLE" not in str(e) and "UNAVAILABLE" not in str(e):
                raise
    raise last


def kernel(x, lut_table, mapping, flip_mask):
    b, inp = x.shape
    nn = lut_table.shape[0]
    nl = nn // NCORES
    key = (nl, b, inp)
    if key not in _CACHE:
        _CACHE[key] = _build_nc(nl, b, inp)
    nc = _CACHE[key]
    in_maps = _prep_core_inputs(x, lut_table, mapping, flip_mask, nl, b, inp)
    res = _run(nc, in_maps)
    outT = np.concatenate([res.results[c]["outT"] for c in range(NCORES)], axis=0)
    return np.ascontiguousarray(outT.T, dtype=np.float32)


# revision 8
# speedup vs baseline: 1.2848x; 1.0964x over previous
"""Trainium2 Bass kernel for BaseLUTLayer (probabilistic LUT node eval).

Math (per reference):
  x_eff = where(flip, 1 - x, x)                      # (B, IN)
  g[b,n,j] = x_eff[b, mapping[n,j]]                  # gather, (B, N, 6)
  out[b,n] = sum_k sigmoid(lut[n,k]) * prod_j (g_j if bit_j(k) else 1-g_j)

Device algorithm (centered-monomial basis):
  host:  t[b,i] = (x[b,i] - 0.5) * (1 - 2*flip[b,i])          (fp16, (IN,B))
         C[n,:] = centered-monomial transform of sigmoid(lut[n,:])
                  (out = sum_S C[n,S] * prod_{j in S} t_j, |t_j| <= 0.5)
  dev:   gather the 6 t-rows per node (dma_gather), then fold:
           level 0:  U[m] = C[2m] + t0 * C[2m+1]     32 scalar-FMA rows
                     (per-partition fp32 scalar operands -> DVE 4x / ACT / Pool)
           level j:  V = U_even + t_j * U_odd        mul+add tensor rows,
                     batch columns split between a DVE lane and a Pool lane.

Sharding: nodes split 8 ways (1024 nodes/core); batch replicated.
Per-core output is (1024, 256) fp32, host concatenates + transposes.
"""

import numpy as np

B = 256
IN = 8192
NN = 8192
FAN = 6
NPAT = 64
NCORES = 8
PT = 128  # nodes per tile (partition dim)

# engine split tuning (see _build_nc)
N_ACT = 20   # level-0 FMA rows on ACT (of 32)
N_POOL = 2   # level-0 FMA rows on Pool
W_DVE = 216  # batch columns of levels 1-5 on DVE (rest on Pool)
NG = 2       # number of gather chunks
BUFS = 3     # work pool depth (pipeline tiles)

_CACHE = {}


def _build_nc(nl, b, inp, n_act=N_ACT, n_pool=N_POOL, w_dve=W_DVE, ng=NG, bufs=BUFS):
    """Build + compile the SPMD Bass program for one core's slice.

    Level-0 row assignment: rows are split DVE-first / ACT / Pool-last, and
    levels 1+ on the DVE lane are chunked in q so the first L1 chunk only
    depends on the early U rows.
    """
    import concourse.bacc as bacc
    import concourse.mybir as mybir
    from concourse.tile import TileContext
    from concourse._compat import get_trn_type

    dt = mybir.dt
    Alu = mybir.AluOpType
    Act = mybir.ActivationFunctionType

    nt = nl // PT              # tiles
    tpg = nt // ng             # tiles per gather
    npg = PT * FAN * tpg       # indices per gather
    iw = npg // 16             # idx wrap columns per gather

    nc = bacc.Bacc(
        get_trn_type() or "TRN2",
        target_bir_lowering=False,
        debug=False,
        num_devices=NCORES,
    )
    tT = nc.dram_tensor("tT", [inp, b], dt.float16, kind="ExternalInput")
    # host-packed: Cpk[p, t*64+k] = C[t*128+p, k]
    Ctab = nc.dram_tensor("C", [128, nt * NPAT], dt.float32, kind="ExternalInput")
    idx = nc.dram_tensor("idx", [128, ng * iw], dt.int16, kind="ExternalInput")
    outT = nc.dram_tensor("outT", [nl, b], dt.float32, kind="ExternalOutput")

    f16, f32 = dt.float16, dt.float32
    wP = b - w_dve
    n_dve = 32 - n_act - n_pool

    # U row -> engine: DVE first (fast, unblocks L1 chunk A), ACT middle, Pool last
    row_eng = ["dve"] * n_dve + ["act"] * n_act + ["pool"] * n_pool

    with TileContext(nc) as tc:
        with (
            tc.tile_pool(name="const", bufs=1) as cpool,
            tc.tile_pool(name="ld", bufs=2) as ld,
            tc.tile_pool(name="work", bufs=bufs) as wk,
        ):
            idx_sb = cpool.tile([128, ng * iw], dt.int16)
            nc.sync.dma_start(idx_sb[:, :], idx[:, :])
            C_sb = cpool.tile([128, nt * NPAT], f32)
            nc.sync.dma_start(C_sb[:, :], Ctab[:, :])

            gt = []
            for G in range(ng):
                g = cpool.tile([128, tpg * FAN, b], f16, tag=f"g{G}")
                nc.gpsimd.dma_gather(
                    g[:, :, :], tT[:, :], idx_sb[:, G * iw:(G + 1) * iw],
                    npg, npg, b,
                )
                gt.append(g)

            for t in range(nt):
                G, tl = divmod(t, tpg)
                a = lambda j: gt[G][:, tl * FAN + j, :]
                a3 = lambda j, sl: gt[G][:, tl * FAN + j:tl * FAN + j + 1, sl]
                Ct = C_sb[:, t * NPAT:(t + 1) * NPAT]

                # --- level 0: U[m] = C[2m] + t0*C[2m+1], 32 scalar-FMA rows ---
                U = wk.tile([128, 32, b], f16, tag="U")
                t0 = a(0)
                for m in range(32):
                    dst = U[:, m, :]
                    sc, bi = Ct[:, 2 * m + 1:2 * m + 2], Ct[:, 2 * m:2 * m + 1]
                    e = row_eng[m]
                    if e == "act":
                        nc.scalar.activation(dst, t0, Act.Identity, scale=sc, bias=bi)
                    elif e == "pool":
                        nc.gpsimd.tensor_scalar(
                            out=dst, in0=t0, scalar1=sc, scalar2=bi,
                            op0=Alu.mult, op1=Alu.add,
                        )
                    else:
                        nc.vector.tensor_scalar(
                            out=dst, in0=t0, scalar1=sc, scalar2=bi,
                            op0=Alu.mult, op1=Alu.add,
                        )

                # --- levels 1..5: V = U_even + t_j*U_odd, two column lanes ---
                out_t = wk.tile([128, 1, b], f32, tag="out")
                lanes = []
                if w_dve > 0:
                    lanes.append((nc.vector, slice(0, w_dve), w_dve, "D", 2))
                if wP > 0:
                    lanes.append((nc.gpsimd, slice(w_dve, b), wP, "P", 1))
                for eng, sl, w, nm, nch in lanes:
                    V = U[:, :, sl]
                    for j in range(1, 6):
                        h = 32 >> j
                        P = wk.tile([128, h, w], f16, tag=f"P{j}{nm}")
                        if j < 5:
                            Vn = wk.tile([128, h, w], f16, tag=f"V{j}{nm}")
                        else:
                            Vn = out_t[:, :, sl]
                        ch = nch if h >= nch * 2 else 1
                        hc = h // ch
                        for c in range(ch):
                            qs = slice(c * hc, (c + 1) * hc)
                            lo, hi = 2 * c * hc, 2 * (c + 1) * hc
                            tjb = a3(j, sl).broadcast_to([128, hc, w])
                            eng.tensor_mul(P[:, qs, :], V[:, lo + 1:hi:2, :], tjb)
                            eng.tensor_add(Vn[:, qs, :], P[:, qs, :], V[:, lo:hi:2, :])
                        if j < 5:
                            V = Vn

                nc.sync.dma_start(outT[t * PT:(t + 1) * PT, :], out_t[:, 0, :])

    nc.compile()
    return nc


def _prep_core_inputs(x, lut_table, mapping, flip_mask, nl, b, inp, n_cores=NCORES, ng=NG):
    """Host-side layout prep: t-table, centered-monomial tables, packed indices."""
    x = np.asarray(x)
    flip = np.asarray(flip_mask)
    # t[b,i] = (x-0.5)*(1-2f), transposed to (IN, B) fp16 for the gather
    tT = np.ascontiguousarray(
        ((x - 0.5) * (1.0 - 2.0 * flip)).T.astype(np.float16)
    )

    # centered-monomial transform of sigmoid(lut): out = sum_S C_S prod_{j in S} t_j
    lut64 = np.asarray(lut_table, dtype=np.float64)
    s = 1.0 / (1.0 + np.exp(-lut64))
    C = s.reshape(-1, 2, 2, 2, 2, 2, 2)  # axes [N, b5, b4, b3, b2, b1, b0]
    for j in range(6):
        ax = 1 + (5 - j)
        e = np.take(C, 0, axis=ax)
        o = np.take(C, 1, axis=ax)
        C = np.stack([0.5 * (e + o), o - e], axis=ax)
    C = C.reshape(-1, NPAT).astype(np.float32)

    nt = nl // PT
    tpg = nt // ng
    in_maps = []
    for c in range(n_cores):
        sl = slice(c * nl, (c + 1) * nl)
        m_c = np.asarray(mapping[sl])  # (nl, 6) int32
        # per gather G: local index j = (tl*6+f)*128 + p -> m_c[(G*tpg+tl)*128+p, f]
        order = m_c.reshape(ng, tpg, PT, FAN).transpose(0, 1, 3, 2).reshape(ng, -1)
        wraps = []
        for G in range(ng):
            w = np.ascontiguousarray(order[G].astype(np.int16).reshape(-1, 16).T)
            wraps.append(np.tile(w, (8, 1)))  # (128, iw)
        idx_full = np.concatenate(wraps, axis=1)
        # pack C: Cpk[p, t*64+k] = C[t*128+p, k]
        Cpk = np.ascontiguousarray(
            C[sl].reshape(nt, PT, NPAT).transpose(1, 0, 2).reshape(PT, nt * NPAT)
        )
        in_maps.append({"tT": tT, "C": Cpk, "idx": idx_full})
    return in_maps


def _run(nc, in_maps, **kw):
    from concourse.bass_utils import run_bass_kernel_spmd

    last = None
    for attempt in range(3):
        try:
            return run_bass_kernel_spmd(nc, in_maps, list(range(NCORES)), **kw)
        except Exception as e:  # transient device errors happen on this fabric
            last = e
            if "UNRECOVERABLE" not in str(e) and "UNAVAILABLE" not in str(e):
                raise
    raise last


def kernel(x, lut_table, mapping, flip_mask):
    b, inp = x.shape
    nn = lut_table.shape[0]
    nl = nn // NCORES
    key = (nl, b, inp)
    if key not in _CACHE:
        _CACHE[key] = _build_nc(nl, b, inp)
    nc = _CACHE[key]
    in_maps = _prep_core_inputs(x, lut_table, mapping, flip_mask, nl, b, inp)
    res = _run(nc, in_maps)
    outT = np.concatenate([res.results[c]["outT"] for c in range(NCORES)], axis=0)
    return np.ascontiguousarray(outT.T, dtype=np.float32)


# revision 11
# speedup vs baseline: 1.3231x; 1.0298x over previous
"""Trainium2 Bass kernel for BaseLUTLayer (probabilistic LUT node eval).

Math (per reference):
  x_eff = where(flip, 1 - x, x)                      # (B, IN)
  g[b,n,j] = x_eff[b, mapping[n,j]]                  # gather, (B, N, 6)
  out[b,n] = sum_k sigmoid(lut[n,k]) * prod_j (g_j if bit_j(k) else 1-g_j)

Device algorithm (centered-monomial basis):
  host:  t[b,i] = (x[b,i] - 0.5) * (1 - 2*flip[b,i])          (fp16, (IN,B))
         C[n,:] = centered-monomial transform of sigmoid(lut[n,:])
                  (out = sum_S C[n,S] * prod_{j in S} t_j, |t_j| <= 0.5)
  dev:   gather the 6 t-rows per node (dma_gather), then fold:
           level 0:  U[m] = C[2m] + t0 * C[2m+1]     32 scalar-FMA rows
                     (per-partition fp32 scalar operands -> DVE 4x / ACT / Pool)
           level j:  V = U_even + t_j * U_odd        mul+add tensor rows,
                     batch columns split between a DVE lane and a Pool lane.

Sharding: nodes split 8 ways (1024 nodes/core); batch replicated.
Per-core output is (1024, 256) fp32, host concatenates + transposes.
"""

import numpy as np

B = 256
IN = 8192
NN = 8192
FAN = 6
NPAT = 64
NCORES = 8
PT = 128  # nodes per tile (partition dim)

# engine split tuning (see _build_nc)
N_ACT = 20        # level-0 FMA rows on ACT (of 32), per tile (int or list)
N_POOL = 2        # level-0 FMA rows on Pool, per tile (int or list)
W_DVE = 212       # batch columns of levels 1-5 on DVE (rest Pool), per tile
GGROUPS = (1, 3, 4)  # tiles per gather chunk
BUFS = 3          # work pool depth (pipeline tiles)
L1_CH = 2         # DVE-lane chunking of level 1

_CACHE = {}


def _per_tile(v, nt):
    return list(v) if isinstance(v, (list, tuple)) else [v] * nt


def _build_nc(nl, b, inp, n_act=N_ACT, n_pool=N_POOL, w_dve=W_DVE,
              ggroups=GGROUPS, bufs=BUFS, l1_ch=L1_CH):
    """Build + compile the SPMD Bass program for one core's slice.

    Level-0 row assignment: rows are split DVE-first / ACT / Pool-last, and
    level 1 on the DVE lane is chunked in q so the first L1 chunk only
    depends on the early U rows.
    """
    import concourse.bacc as bacc
    import concourse.mybir as mybir
    from concourse.tile import TileContext
    from concourse._compat import get_trn_type

    dt = mybir.dt
    Alu = mybir.AluOpType
    Act = mybir.ActivationFunctionType

    nt = nl // PT              # tiles
    assert sum(ggroups) == nt
    n_act = _per_tile(n_act, nt)
    n_pool = _per_tile(n_pool, nt)
    w_dve = _per_tile(w_dve, nt)

    nc = bacc.Bacc(
        get_trn_type() or "TRN2",
        target_bir_lowering=False,
        debug=False,
        num_devices=NCORES,
    )
    tT = nc.dram_tensor("tT", [inp, b], dt.float16, kind="ExternalInput")
    # host-packed: Cpk[p, t*64+k] = C[t*128+p, k]
    Ctab = nc.dram_tensor("C", [128, nt * NPAT], dt.float32, kind="ExternalInput")
    n_idx = nl * FAN
    idx = nc.dram_tensor("idx", [128, n_idx // 16], dt.int16, kind="ExternalInput")
    outT = nc.dram_tensor("outT", [nl, b], dt.float32, kind="ExternalOutput")

    f16, f32 = dt.float16, dt.float32

    with TileContext(nc) as tc:
        with (
            tc.tile_pool(name="const", bufs=1) as cpool,
            tc.tile_pool(name="work", bufs=bufs) as wk,
        ):
            idx_sb = cpool.tile([128, n_idx // 16], dt.int16)
            nc.sync.dma_start(idx_sb[:, :], idx[:, :])
            C_sb = cpool.tile([128, nt * NPAT], f32)
            nc.sync.dma_start(C_sb[:, :], Ctab[:, :])

            # gathers: ggroups[G] tiles each; tile t -> (gather G, local tile tl)
            gt, t2g = [], {}
            t0i = 0
            iw0 = 0
            for G, tg in enumerate(ggroups):
                npg = PT * FAN * tg
                iw = npg // 16
                g = cpool.tile([128, tg * FAN, b], f16, tag=f"g{G}")
                nc.gpsimd.dma_gather(
                    g[:, :, :], tT[:, :], idx_sb[:, iw0:iw0 + iw],
                    npg, npg, b,
                )
                gt.append(g)
                for tl in range(tg):
                    t2g[t0i + tl] = (G, tl)
                t0i += tg
                iw0 += iw

            for t in range(nt):
                G, tl = t2g[t]
                a = lambda j: gt[G][:, tl * FAN + j, :]
                a3 = lambda j, sl: gt[G][:, tl * FAN + j:tl * FAN + j + 1, sl]
                Ct = C_sb[:, t * NPAT:(t + 1) * NPAT]
                nA, nP, wD = n_act[t], n_pool[t], w_dve[t]
                nD = 32 - nA - nP
                row_eng = ["dve"] * nD + ["act"] * nA + ["pool"] * nP

                # --- level 0: U[m] = C[2m] + t0*C[2m+1], 32 scalar-FMA rows ---
                U = wk.tile([128, 32, b], f16, tag="U")
                t0 = a(0)
                for m in range(32):
                    dst = U[:, m, :]
                    sc, bi = Ct[:, 2 * m + 1:2 * m + 2], Ct[:, 2 * m:2 * m + 1]
                    e = row_eng[m]
                    if e == "act":
                        nc.scalar.activation(dst, t0, Act.Identity, scale=sc, bias=bi)
                    elif e == "pool":
                        nc.gpsimd.tensor_scalar(
                            out=dst, in0=t0, scalar1=sc, scalar2=bi,
                            op0=Alu.mult, op1=Alu.add,
                        )
                    else:
                        nc.vector.tensor_scalar(
                            out=dst, in0=t0, scalar1=sc, scalar2=bi,
                            op0=Alu.mult, op1=Alu.add,
                        )

                # --- levels 1..5: V = U_even + t_j*U_odd, two column lanes ---
                out_t = wk.tile([128, 1, b], f32, tag="out")
                lanes = []
                if wD > 0:
                    lanes.append((nc.vector, slice(0, wD), wD, "D"))
                if wD < b:
                    lanes.append((nc.gpsimd, slice(wD, b), b - wD, "P"))
                for eng, sl, w, nm in lanes:
                    V = U[:, :, sl]
                    for j in range(1, 6):
                        h = 32 >> j
                        P = wk.tile([128, h, w], f16, tag=f"P{j}{nm}")
                        if j < 5:
                            Vn = wk.tile([128, h, w], f16, tag=f"V{j}{nm}")
                        else:
                            Vn = out_t[:, :, sl]
                        ch = l1_ch if (j == 1 and nm == "D") else 1
                        hc = h // ch
                        for c in range(ch):
                            qs = slice(c * hc, (c + 1) * hc)
                            lo, hi = 2 * c * hc, 2 * (c + 1) * hc
                            tjb = a3(j, sl).broadcast_to([128, hc, w])
                            eng.tensor_mul(P[:, qs, :], V[:, lo + 1:hi:2, :], tjb)
                            eng.tensor_add(Vn[:, qs, :], P[:, qs, :], V[:, lo:hi:2, :])
                        if j < 5:
                            V = Vn

                nc.sync.dma_start(outT[t * PT:(t + 1) * PT, :], out_t[:, 0, :])

    nc.compile()
    return nc


def _prep_core_inputs(x, lut_table, mapping, flip_mask, nl, b, inp, n_cores=NCORES, ggroups=GGROUPS):
    """Host-side layout prep: t-table, centered-monomial tables, packed indices."""
    x = np.asarray(x)
    flip = np.asarray(flip_mask)
    # t[b,i] = (x-0.5)*(1-2f), transposed to (IN, B) fp16 for the gather
    tT = np.ascontiguousarray(
        ((x - 0.5) * (1.0 - 2.0 * flip)).T.astype(np.float16)
    )

    # centered-monomial transform of sigmoid(lut): out = sum_S C_S prod_{j in S} t_j
    lut64 = np.asarray(lut_table, dtype=np.float64)
    s = 1.0 / (1.0 + np.exp(-lut64))
    C = s.reshape(-1, 2, 2, 2, 2, 2, 2)  # axes [N, b5, b4, b3, b2, b1, b0]
    for j in range(6):
        ax = 1 + (5 - j)
        e = np.take(C, 0, axis=ax)
        o = np.take(C, 1, axis=ax)
        C = np.stack([0.5 * (e + o), o - e], axis=ax)
    C = C.reshape(-1, NPAT).astype(np.float32)

    nt = nl // PT
    in_maps = []
    for c in range(n_cores):
        sl = slice(c * nl, (c + 1) * nl)
        m_c = np.asarray(mapping[sl])  # (nl, 6) int32
        # per gather G (covering tiles t0..t0+tg-1):
        #   local index j = (tl*6+f)*128 + p -> m_c[(t0+tl)*128+p, f]
        by_tile = m_c.reshape(nt, PT, FAN).transpose(0, 2, 1)  # (nt, FAN, PT)
        wraps = []
        t0i = 0
        for tg in ggroups:
            og = by_tile[t0i:t0i + tg].reshape(-1)
            w = np.ascontiguousarray(og.astype(np.int16).reshape(-1, 16).T)
            wraps.append(np.tile(w, (8, 1)))  # (128, iw)
            t0i += tg
        idx_full = np.concatenate(wraps, axis=1)
        # pack C: Cpk[p, t*64+k] = C[t*128+p, k]
        Cpk = np.ascontiguousarray(
            C[sl].reshape(nt, PT, NPAT).transpose(1, 0, 2).reshape(PT, nt * NPAT)
        )
        in_maps.append({"tT": tT, "C": Cpk, "idx": idx_full})
    return in_maps


def _run(nc, in_maps, **kw):
    from concourse.bass_utils import run_bass_kernel_spmd

    last = None
    for attempt in range(3):
        try:
            return run_bass_kernel_spmd(nc, in_maps, list(range(NCORES)), **kw)
        except Exception as e:  # transient device errors happen on this fabric
            last = e
            if "UNRECOVERABLE" not in str(e) and "UNAVAILABLE" not in str(e):
                raise
    raise last


def kernel(x, lut_table, mapping, flip_mask):
    b, inp = x.shape
    nn = lut_table.shape[0]
    nl = nn // NCORES
    key = (nl, b, inp)
    if key not in _CACHE:
        _CACHE[key] = _build_nc(nl, b, inp)
    nc = _CACHE[key]
    in_maps = _prep_core_inputs(x, lut_table, mapping, flip_mask, nl, b, inp)
    res = _run(nc, in_maps)
    outT = np.concatenate([res.results[c]["outT"] for c in range(NCORES)], axis=0)
    return np.ascontiguousarray(outT.T, dtype=np.float32)


# revision 14
# speedup vs baseline: 1.4329x; 1.0830x over previous
"""Trainium2 Bass kernel for BaseLUTLayer (probabilistic LUT node eval).

Math (per reference):
  x_eff = where(flip, 1 - x, x)                      # (B, IN)
  g[b,n,j] = x_eff[b, mapping[n,j]]                  # gather, (B, N, 6)
  out[b,n] = sum_k sigmoid(lut[n,k]) * prod_j (g_j if bit_j(k) else 1-g_j)

Device algorithm (centered-monomial basis):
  host:  t[b,i] = (x[b,i] - 0.5) * (1 - 2*flip[b,i])          (fp16, (IN,B))
         C[n,:] = centered-monomial transform of sigmoid(lut[n,:])
                  (out = sum_S C[n,S] * prod_{j in S} t_j, |t_j| <= 0.5)
  dev:   gather the 6 t-rows per node (dma_gather), then fold:
           level 0:  U[m] = C[2m] + t0 * C[2m+1]     32 scalar-FMA rows
                     (per-partition fp32 scalar operands -> DVE 4x / ACT / Pool)
           level j:  V = U_even + t_j * U_odd        mul+add tensor rows,
                     batch columns split between a DVE lane and a Pool lane.

Sharding: nodes split 8 ways (1024 nodes/core); batch replicated.
Per-core output is (1024, 256) fp32, host concatenates + transposes.
"""

import numpy as np

B = 256
IN = 8192
NN = 8192
FAN = 6
NPAT = 64
NCORES = 8
PT = 128  # nodes per tile (partition dim)

# engine split tuning (see _build_nc)
N_ACT = 20        # level-0 FMA rows on ACT (of 32), per tile (int or list)
N_POOL = 2        # level-0 FMA rows on Pool, per tile (int or list)
W_DVE = 212       # batch columns of levels 1-5 on DVE (rest Pool), per tile
GGROUPS = (1, 3, 4)  # tiles per gather chunk
BUFS = 3          # work pool depth (pipeline tiles)
L1_CH = 2         # DVE-lane chunking of level 1

_CACHE = {}


def _per_tile(v, nt):
    return list(v) if isinstance(v, (list, tuple)) else [v] * nt


def _build_nc(nl, b, inp, n_act=N_ACT, n_pool=N_POOL, w_dve=W_DVE,
              ggroups=GGROUPS, bufs=BUFS, l1_ch=L1_CH):
    """Build + compile the SPMD Bass program for one core's slice.

    Level-0 row assignment: rows are split DVE-first / ACT / Pool-last, and
    level 1 on the DVE lane is chunked in q so the first L1 chunk only
    depends on the early U rows.
    """
    import concourse.bacc as bacc
    import concourse.mybir as mybir
    from concourse.tile import TileContext
    from concourse._compat import get_trn_type

    dt = mybir.dt
    Alu = mybir.AluOpType
    Act = mybir.ActivationFunctionType

    nt = nl // PT              # tiles
    assert sum(ggroups) == nt
    n_act = _per_tile(n_act, nt)
    n_pool = _per_tile(n_pool, nt)
    w_dve = _per_tile(w_dve, nt)

    nc = bacc.Bacc(
        get_trn_type() or "TRN2",
        target_bir_lowering=False,
        debug=False,
        num_devices=NCORES,
    )
    tT = nc.dram_tensor("tT", [inp, b], dt.float16, kind="ExternalInput")
    # host-packed: Cpk[p, t*64+k] = C[t*128+p, k]
    Ctab = nc.dram_tensor("C", [128, nt * NPAT], dt.float32, kind="ExternalInput")
    n_idx = nl * FAN
    idx = nc.dram_tensor("idx", [128, n_idx // 16], dt.int16, kind="ExternalInput")
    outT = nc.dram_tensor("outT", [nl, b], dt.float32, kind="ExternalOutput")

    f16, f32 = dt.float16, dt.float32

    with TileContext(nc) as tc:
        with (
            tc.tile_pool(name="const", bufs=1) as cpool,
            tc.tile_pool(name="upool", bufs=bufs + 1) as up,
            tc.tile_pool(name="work", bufs=bufs) as wk,
        ):
            idx_sb = cpool.tile([128, n_idx // 16], dt.int16)
            nc.sync.dma_start(idx_sb[:, :], idx[:, :])
            C_sb = cpool.tile([128, nt * NPAT], f32)
            nc.sync.dma_start(C_sb[:, :], Ctab[:, :])

            # warm the ACT function table before real work (1.3us load)
            warm = cpool.tile([128, 2], f16)
            nc.vector.memset(warm[:, :], 0.0)
            nc.scalar.activation(warm[:, :], warm[:, :], Act.Identity)

            # gathers: ggroups[G] tiles each; tile t -> (gather G, local tile tl)
            gt, t2g = [], {}
            t0i = 0
            iw0 = 0
            for G, tg in enumerate(ggroups):
                npg = PT * FAN * tg
                iw = npg // 16
                g = cpool.tile([128, tg * FAN, b], f16, tag=f"g{G}")
                nc.gpsimd.dma_gather(
                    g[:, :, :], tT[:, :], idx_sb[:, iw0:iw0 + iw],
                    npg, npg, b,
                )
                gt.append(g)
                for tl in range(tg):
                    t2g[t0i + tl] = (G, tl)
                t0i += tg
                iw0 += iw

            for t in range(nt):
                G, tl = t2g[t]
                a = lambda j: gt[G][:, tl * FAN + j, :]
                a3 = lambda j, sl: gt[G][:, tl * FAN + j:tl * FAN + j + 1, sl]
                Ct = C_sb[:, t * NPAT:(t + 1) * NPAT]
                nA, nP, wD = n_act[t], n_pool[t], w_dve[t]
                nD = 32 - nA - nP
                row_eng = ["dve"] * nD + ["act"] * nA + ["pool"] * nP

                # --- level 0: U[m] = C[2m] + t0*C[2m+1], 32 scalar-FMA rows ---
                U = up.tile([128, 32, b], f16, tag="U")
                t0 = a(0)
                for m in range(32):
                    dst = U[:, m, :]
                    sc, bi = Ct[:, 2 * m + 1:2 * m + 2], Ct[:, 2 * m:2 * m + 1]
                    e = row_eng[m]
                    if e == "act":
                        nc.scalar.activation(dst, t0, Act.Identity, scale=sc, bias=bi)
                    elif e == "pool":
                        nc.gpsimd.tensor_scalar(
                            out=dst, in0=t0, scalar1=sc, scalar2=bi,
                            op0=Alu.mult, op1=Alu.add,
                        )
                    else:
                        nc.vector.tensor_scalar(
                            out=dst, in0=t0, scalar1=sc, scalar2=bi,
                            op0=Alu.mult, op1=Alu.add,
                        )

                # --- levels 1..5: V = U_even + t_j*U_odd, two column lanes ---
                out_t = wk.tile([128, 1, b], f32, tag="out")
                lanes = []
                if wD > 0:
                    lanes.append((nc.vector, slice(0, wD), wD, "D"))
                if wD < b:
                    lanes.append((nc.gpsimd, slice(wD, b), b - wD, "P"))
                for eng, sl, w, nm in lanes:
                    V = U[:, :, sl]
                    pool_lane = nm == "P"
                    for j in range(1, 6):
                        h = 32 >> j
                        P = wk.tile([128, h, w], f16, tag=f"P{j}{nm}")
                        if j < 5:
                            Vn = wk.tile([128, h, w], f16, tag=f"V{j}{nm}")
                        else:
                            Vn = out_t[:, :, sl]
                        ch = l1_ch if (j == 1 and nm == "D") else 1
                        hc = h // ch
                        for c in range(ch):
                            qs = slice(c * hc, (c + 1) * hc)
                            lo, hi = 2 * c * hc, 2 * (c + 1) * hc
                            tjb = a3(j, sl).broadcast_to([128, hc, w])
                            if pool_lane:
                                # TensorScalarPtr path: 0.6 gpsimd efficiency
                                # vs 0.42 for plain TensorTensor
                                eng.scalar_tensor_tensor(
                                    P[:, qs, :], V[:, lo + 1:hi:2, :], 1.0, tjb,
                                    Alu.bypass, Alu.mult,
                                )
                                eng.scalar_tensor_tensor(
                                    Vn[:, qs, :], P[:, qs, :], 1.0, V[:, lo:hi:2, :],
                                    Alu.bypass, Alu.add,
                                )
                            else:
                                eng.tensor_mul(P[:, qs, :], V[:, lo + 1:hi:2, :], tjb)
                                eng.tensor_add(Vn[:, qs, :], P[:, qs, :], V[:, lo:hi:2, :])
                        if j < 5:
                            V = Vn

                nc.sync.dma_start(outT[t * PT:(t + 1) * PT, :], out_t[:, 0, :])

    nc.compile()
    return nc


def _prep_core_inputs(x, lut_table, mapping, flip_mask, nl, b, inp, n_cores=NCORES, ggroups=GGROUPS):
    """Host-side layout prep: t-table, centered-monomial tables, packed indices."""
    x = np.asarray(x)
    flip = np.asarray(flip_mask)
    # t[b,i] = (x-0.5)*(1-2f), transposed to (IN, B) fp16 for the gather
    tT = np.ascontiguousarray(
        ((x - 0.5) * (1.0 - 2.0 * flip)).T.astype(np.float16)
    )

    # centered-monomial transform of sigmoid(lut): out = sum_S C_S prod_{j in S} t_j
    lut64 = np.asarray(lut_table, dtype=np.float64)
    s = 1.0 / (1.0 + np.exp(-lut64))
    C = s.reshape(-1, 2, 2, 2, 2, 2, 2)  # axes [N, b5, b4, b3, b2, b1, b0]
    for j in range(6):
        ax = 1 + (5 - j)
        e = np.take(C, 0, axis=ax)
        o = np.take(C, 1, axis=ax)
        C = np.stack([0.5 * (e + o), o - e], axis=ax)
    C = C.reshape(-1, NPAT).astype(np.float32)

    nt = nl // PT
    in_maps = []
    for c in range(n_cores):
        sl = slice(c * nl, (c + 1) * nl)
        m_c = np.asarray(mapping[sl])  # (nl, 6) int32
        # per gather G (covering tiles t0..t0+tg-1):
        #   local index j = (tl*6+f)*128 + p -> m_c[(t0+tl)*128+p, f]
        by_tile = m_c.reshape(nt, PT, FAN).transpose(0, 2, 1)  # (nt, FAN, PT)
        wraps = []
        t0i = 0
        for tg in ggroups:
            og = by_tile[t0i:t0i + tg].reshape(-1)
            w = np.ascontiguousarray(og.astype(np.int16).reshape(-1, 16).T)
            wraps.append(np.tile(w, (8, 1)))  # (128, iw)
            t0i += tg
        idx_full = np.concatenate(wraps, axis=1)
        # pack C: Cpk[p, t*64+k] = C[t*128+p, k]
        Cpk = np.ascontiguousarray(
            C[sl].reshape(nt, PT, NPAT).transpose(1, 0, 2).reshape(PT, nt * NPAT)
        )
        in_maps.append({"tT": tT, "C": Cpk, "idx": idx_full})
    return in_maps


def _run(nc, in_maps, **kw):
    from concourse.bass_utils import run_bass_kernel_spmd

    last = None
    for attempt in range(3):
        try:
            return run_bass_kernel_spmd(nc, in_maps, list(range(NCORES)), **kw)
        except Exception as e:  # transient device errors happen on this fabric
            last = e
            if "UNRECOVERABLE" not in str(e) and "UNAVAILABLE" not in str(e):
                raise
    raise last


def kernel(x, lut_table, mapping, flip_mask):
    b, inp = x.shape
    nn = lut_table.shape[0]
    nl = nn // NCORES
    key = (nl, b, inp)
    if key not in _CACHE:
        _CACHE[key] = _build_nc(nl, b, inp)
    nc = _CACHE[key]
    in_maps = _prep_core_inputs(x, lut_table, mapping, flip_mask, nl, b, inp)
    res = _run(nc, in_maps)
    outT = np.concatenate([res.results[c]["outT"] for c in range(NCORES)], axis=0)
    return np.ascontiguousarray(outT.T, dtype=np.float32)
